# revision 15
# baseline (speedup 1.0000x reference)
"""Trainium2 Bass kernel: batched Euler-Maruyama integration of a neural SDE.

Reference computation (per step t):
    vf     = -y + MLP(y)          MLP: tanh(64->256), tanh(256->256) x2, 256->64
    y_next = y + dt_t * vf + SIGMA * sqrt(dt_t) * dW_t
Output: all intermediate states [T+1, B, D].

Strategy
--------
Data-parallel over the particle axis: B=4096 -> 512 particles on each of the
8 NeuronCores.  All math runs in a feature-major ("transposed") layout
Y.T [D, B_local] so every MLP layer is a plain stationary-weight matmul with
the contraction on the partition axis -- no on-device transposes at all:

  * host pre-transposes y0 and the (pre-scaled) noise to [.., D, B_local]
  * PE:  H1 = Win @ Y, H2 = W0 @ H1, H3 = W1 @ H2 (K-chunked, M-folded into
         one PSUM tile per layer), vf_psum = Wout @ H3 - Y (identity matmul)
  * ACT: tanh epilogues straight out of PSUM (one instr per layer when the
         biases are zero, per-M-tile with per-partition bias otherwise)
  * DVE: y_next = (vf_psum * dt_t + noise_t) + y  (2 ops)
  * per-step DMA: noise tile in, state tile out (contiguous, pre-transposed)
  * matmuls run as float32r (full-rate fp32 streaming mode)

The batch is split into NCHUNK chunks so PE matmuls of one chunk overlap ACT
tanh of the other; steps are serially dependent so this is the only
parallelism axis inside a core.

The host prepends y0, transposes outputs back and concatenates the 8 shards.
"""

import numpy as np

B, D, W, T = 4096, 64, 256, 256
NCORES = 8
BL = B // NCORES  # 512 particles per core
SIGMA = 0.1

NCHUNK = 2        # batch chunks per core, interleaved for PE/ACT overlap


def _build(dts, zero_bias, nchunk=NCHUNK, steps=T, bl=BL):
    """Emit the Bass/Tile program. dts: numpy [steps] fp32 per-step dt.

    Matmul operands are fp16 (1 cycle/row on the PE + fast weight load);
    the integration state y stays fp32 (a gpsimd fp16 shadow copy feeds the
    matmuls), so state error does not accumulate beyond fp16 MLP noise that
    is scaled by dt each step.
    """
    import concourse.bass as bass  # noqa: F401
    import concourse.mybir as mybir
    import concourse.tile as tile
    from concourse import bacc

    f32 = mybir.dt.float32
    f16 = mybir.dt.float16
    Tanh = mybir.ActivationFunctionType.Tanh
    MULT = mybir.AluOpType.mult
    ADD = mybir.AluOpType.add

    csz = bl // nchunk
    assert csz * nchunk == bl
    chunks = list(range(nchunk))

    nc = bacc.Bacc("TRN2", target_bir_lowering=False, debug=False)

    y0_d = nc.dram_tensor("y0t", [D, bl], f32, kind="ExternalInput")
    nz_d = nc.dram_tensor("nz", [steps, D, bl], f32, kind="ExternalInput")
    win_d = nc.dram_tensor("wint", [D, W], f16, kind="ExternalInput")
    wh_d = nc.dram_tensor("wht", [2, 2, 128, W], f16, kind="ExternalInput")
    wout_d = nc.dram_tensor("woutt", [2, 128, D], f16, kind="ExternalInput")
    if not zero_bias:
        bias_d = nc.dram_tensor("biases", [3, 128, 2], f32, kind="ExternalInput")
    out_d = nc.dram_tensor("outt", [steps, D, bl], f32, kind="ExternalOutput")

    mm = nc.tensor.matmul

    with tile.TileContext(nc) as tc:
        with (
            tc.tile_pool(name="const", bufs=1) as const,
            tc.tile_pool(name="hbuf", bufs=2) as hbuf,
            tc.tile_pool(name="state", bufs=3) as st,
            tc.tile_pool(name="nzp", bufs=4) as nzp,
            tc.tile_pool(name="psum", bufs=1, space="PSUM") as ps,
        ):
            # ---- constants (loaded once) ----
            win_s = const.tile([D, W], f16)            # W_in.T  [64, 256]
            nc.sync.dma_start(out=win_s[:], in_=win_d[:])
            wh_s = const.tile([128, 2, 2, W], f16)     # hidden lhsT chunks
            for li in range(2):
                for k in range(2):
                    nc.sync.dma_start(out=wh_s[:, li, k, :], in_=wh_d[li, k])
            wout_s = const.tile([128, 2, D], f16)      # W_out.T chunks
            for k in range(2):
                nc.sync.dma_start(out=wout_s[:, k, :], in_=wout_d[k])
            if not zero_bias:
                bias_s = const.tile([128, 3, 2], f32)
                for j in range(3):
                    nc.sync.dma_start(out=bias_s[:, j, :], in_=bias_d[j])

            # ---- initial state: y (fp32) + fp16 shadow for matmuls ----
            ycur, yhcur = [], []
            for c in chunks:
                y_t = st.tile([D, csz], f32, tag=f"y{c}")
                nc.sync.dma_start(out=y_t[:], in_=y0_d[:, c * csz:(c + 1) * csz])
                yh_t = st.tile([D, csz], f16, tag=f"yh{c}")
                nc.gpsimd.tensor_copy(out=yh_t[:], in_=y_t[:])
                ycur.append(y_t)
                yhcur.append(yh_t)

            def tanh_layer(h_sb, h_ps, li):
                if zero_bias:
                    nc.scalar.activation(out=h_sb[:, :, :], in_=h_ps[:, :, :],
                                         func=Tanh)
                else:
                    for m in range(2):
                        nc.scalar.activation(
                            out=h_sb[:, m, :], in_=h_ps[:, m, :], func=Tanh,
                            bias=bias_s[:, li, m:m + 1], scale=1.0)

            # ---- time stepping (chunks interleaved per stage so each
            #      engine's in-order stream never stalls on the other's) ----
            for t in range(steps):
                dt = float(dts[t])
                nz_t, h1p, h1, h2p, h2, h3p, h3, yp = ({} for _ in range(8))
                for c in chunks:
                    lo = c * csz
                    nz_t[c] = nzp.tile([D, csz], f32, tag=f"nz{c}", name=f"nz_{c}")
                    nc.sync.dma_start(out=nz_t[c][:], in_=nz_d[t, :, lo:lo + csz])

                for c in chunks:   # layer 1: [64 -> 256]
                    h1p[c] = ps.tile([128, 2, csz], f32, tag=f"h1p{c}", name=f"h1p_{c}")
                    mm(h1p[c][:, 0, :], win_s[:, 0:128], yhcur[c][:],
                       start=True, stop=True)
                    mm(h1p[c][:, 1, :], win_s[:, 128:256], yhcur[c][:],
                       start=True, stop=True)
                for c in chunks:
                    h1[c] = hbuf.tile([128, 2, csz], f16, tag=f"h1{c}", name=f"h1_{c}")
                    tanh_layer(h1[c], h1p[c], 0)

                for li, (hp, hs, hprev) in enumerate(
                        ((h2p, h2, h1), (h3p, h3, h2))):
                    for c in chunks:   # hidden layer: [256 -> 256]
                        hp[c] = ps.tile([128, 2, csz], f32, tag=f"h{li + 2}p{c}", name=f"h{li + 2}p_{c}")
                        for m in range(2):
                            for k in range(2):
                                mm(hp[c][:, m, :],
                                   wh_s[:, li, k, m * 128:(m + 1) * 128],
                                   hprev[c][:, k, :], start=(k == 0),
                                   stop=(k == 1))
                    for c in chunks:
                        hs[c] = hbuf.tile([128, 2, csz], f16, tag=f"h{li + 2}{c}", name=f"h{li + 2}_{c}")
                        tanh_layer(hs[c], hp[c], li + 1)

                for c in chunks:   # output layer: vf_psum = Wout @ H3
                    yp[c] = ps.tile([D, csz], f32, tag=f"yp{c}", name=f"yp_{c}")
                    mm(yp[c][:], wout_s[:, 0, :], h3[c][:, 0, :],
                       start=True, stop=False)
                    mm(yp[c][:], wout_s[:, 1, :], h3[c][:, 1, :],
                       start=False, stop=True)

                for c in chunks:
                    lo = c * csz
                    # u = dt * vf_psum + noise ; y' = (1-dt) * y + u
                    u_t = st.tile([D, csz], f32, tag=f"u{c}")
                    nc.vector.scalar_tensor_tensor(
                        out=u_t[:], in0=yp[c][:], scalar=dt, in1=nz_t[c][:],
                        op0=MULT, op1=ADD)
                    y_nx = st.tile([D, csz], f32, tag=f"y{c}")
                    nc.vector.scalar_tensor_tensor(
                        out=y_nx[:], in0=ycur[c][:], scalar=1.0 - dt, in1=u_t[:],
                        op0=MULT, op1=ADD)
                    yh_nx = st.tile([D, csz], f16, tag=f"yh{c}")
                    nc.gpsimd.tensor_copy(out=yh_nx[:], in_=y_nx[:])
                    nc.sync.dma_start(out=out_d[t, :, lo:lo + csz], in_=y_nx[:])
                    ycur[c] = y_nx
                    yhcur[c] = yh_nx
    nc.compile()
    return nc


def _host_prep(ts, y0, dW, w_in, b_in, w_h, b_h, w_out, b_out):
    """Compute per-core input maps + build params. Returns (in_maps, dts, zero_bias)."""
    f = np.float32
    ts = np.asarray(ts, f)
    dts = (ts[1:] - ts[:-1]).astype(f)
    assert dts.shape[0] == T

    zero_bias = (not np.any(b_in)) and (not np.any(b_h))

    # noise folded with its scale (and b_out drift term): sigma*sqrt(dt)*dW + dt*b_out
    scale = (SIGMA * np.sqrt(dts)).astype(f)  # [T]
    drift = (dts[:, None] * np.asarray(b_out, f)[None, :]).astype(f)  # [T, D]

    h = np.float16
    w_inT = np.ascontiguousarray(np.asarray(w_in, f).T.astype(h))    # [64, 256]
    whT = np.ascontiguousarray(
        np.stack([np.asarray(w_h[i], f).T.reshape(2, 128, W) for i in range(2)])
    ).astype(h)                                                      # [2, 2, 128, 256]
    w_outT = np.ascontiguousarray(np.asarray(w_out, f).T.reshape(2, 128, D)).astype(h)
    biases = np.zeros((3, 128, 2), f)
    biases[0] = np.asarray(b_in, f).reshape(2, 128).T
    biases[1] = np.asarray(b_h[0], f).reshape(2, 128).T
    biases[2] = np.asarray(b_h[1], f).reshape(2, 128).T

    y0 = np.asarray(y0, f)
    dW = np.asarray(dW, f)

    in_maps = []
    for c in range(NCORES):
        lo = c * BL
        nzc = dW[:, lo:lo + BL, :] * scale[:, None, None] + drift[:, None, :]
        nzc = np.ascontiguousarray(nzc.transpose(0, 2, 1)).astype(f)  # [T, 64, BL]
        m = {
            "y0t": np.ascontiguousarray(y0[lo:lo + BL].T),
            "nz": nzc,
            "wint": w_inT,
            "wht": whT,
            "woutt": w_outT,
        }
        if not zero_bias:
            m["biases"] = biases
        in_maps.append(m)
    return in_maps, dts, zero_bias


_NC_CACHE = {}

# test-harness hooks (kernel() ignores these unless set by test code)
TRACE = False
LAST_RESULT = None


def kernel(ts, y0, dW, w_in, b_in, w_h, b_h, w_out, b_out):
    global LAST_RESULT
    from concourse.bass_utils import run_bass_kernel_spmd

    in_maps, dts, zero_bias = _host_prep(
        ts, y0, dW, w_in, b_in, w_h, b_h, w_out, b_out)

    key = (zero_bias, np.asarray(dts).tobytes())
    nc = _NC_CACHE.get(key)
    if nc is None:
        nc = _build(dts, zero_bias)
        _NC_CACHE[key] = nc

    res = run_bass_kernel_spmd(nc, in_maps, core_ids=list(range(NCORES)),
                               trace=TRACE)
    LAST_RESULT = res

    out = np.empty((T + 1, B, D), np.float32)
    out[0] = np.asarray(y0, np.float32)
    for c in range(NCORES):
        lo = c * BL
        out[1:, lo:lo + BL, :] = res.results[c]["outt"].transpose(0, 2, 1)
    return out


# revision 20
# speedup vs baseline: 1.5911x; 1.5911x over previous
"""Trainium2 Bass kernel: batched Euler-Maruyama integration of a neural SDE.

Reference computation (per step t):
    vf     = -y + MLP(y)          MLP: tanh(64->256), tanh(256->256) x2, 256->64
    y_next = y + dt_t * vf + SIGMA * sqrt(dt_t) * dW_t
Output: all intermediate states [T+1, B, D].

Strategy
--------
Data-parallel over the particle axis: B=4096 -> 512 particles on each of the
8 NeuronCores.  All math runs in a feature-major ("transposed") layout
Y.T [D, B_local] so every MLP layer is a plain stationary-weight matmul with
the contraction on the partition axis -- no on-device transposes at all:

  * host pre-transposes y0 and the (pre-scaled) noise to [.., D, B_local]
  * PE:  H1 = Win @ Y, H2 = W0 @ H1, H3 = W1 @ H2 (K-chunked, M-folded into
         one PSUM tile per layer), vf_psum = Wout @ H3 - Y (identity matmul)
  * ACT: tanh epilogues straight out of PSUM (one instr per layer when the
         biases are zero, per-M-tile with per-partition bias otherwise)
  * DVE: y_next = (vf_psum * dt_t + noise_t) + y  (2 ops)
  * per-step DMA: noise tile in, state tile out (contiguous, pre-transposed)
  * matmuls run as float32r (full-rate fp32 streaming mode)

The batch is split into NCHUNK chunks so PE matmuls of one chunk overlap ACT
tanh of the other; steps are serially dependent so this is the only
parallelism axis inside a core.

The host prepends y0, transposes outputs back and concatenates the 8 shards.
"""

import numpy as np

B, D, W, T = 4096, 64, 256, 256
NCORES = 8
BL = B // NCORES  # 512 particles per core
SIGMA = 0.1

NCHUNK = 2        # batch chunks per core, interleaved for PE/ACT overlap


def _build(dts, zero_bias, nchunk=NCHUNK, steps=T, bl=BL):
    """Emit the Bass/Tile program. dts: numpy [steps] fp32 per-step dt.

    Matmul operands are fp16 (1 cycle/row on the PE + fast weight load);
    the integration state y stays fp32 (a gpsimd fp16 shadow copy feeds the
    matmuls), so state error does not accumulate beyond fp16 MLP noise that
    is scaled by dt each step.
    """
    import concourse.bass as bass  # noqa: F401
    import concourse.mybir as mybir
    import concourse.tile as tile
    from concourse import bacc

    f32 = mybir.dt.float32
    f16 = mybir.dt.float16
    Tanh = mybir.ActivationFunctionType.Tanh
    MULT = mybir.AluOpType.mult
    ADD = mybir.AluOpType.add

    csz = bl // nchunk
    assert csz * nchunk == bl
    chunks = list(range(nchunk))

    nc = bacc.Bacc("TRN2", target_bir_lowering=False, debug=False)

    y0_d = nc.dram_tensor("y0t", [D, bl], f32, kind="ExternalInput")
    y0h_d = nc.dram_tensor("y0th", [D, bl], f16, kind="ExternalInput")
    nz_d = nc.dram_tensor("nz", [steps, D, bl], f32, kind="ExternalInput")
    win_d = nc.dram_tensor("wint", [D, W], f16, kind="ExternalInput")
    wh_d = nc.dram_tensor("wht", [2, 2, 128, W], f16, kind="ExternalInput")
    wout_d = nc.dram_tensor("woutt", [2, 128, D], f16, kind="ExternalInput")
    if not zero_bias:
        bias_d = nc.dram_tensor("biases", [3, 128, 2], f32, kind="ExternalInput")
    out_d = nc.dram_tensor("outt", [steps, D, bl], f32, kind="ExternalOutput")

    mm = nc.tensor.matmul

    with tile.TileContext(nc) as tc:
        with (
            tc.tile_pool(name="const", bufs=1) as const,
            tc.tile_pool(name="hbuf", bufs=2) as hbuf,
            tc.tile_pool(name="state", bufs=3) as st,
            tc.tile_pool(name="nzp", bufs=4) as nzp,
            tc.tile_pool(name="psum", bufs=1, space="PSUM") as ps,
        ):
            # ---- constants (loaded once) ----
            win_s = const.tile([D, W], f16)            # W_in.T  [64, 256]
            nc.sync.dma_start(out=win_s[:], in_=win_d[:])
            wh_s = const.tile([128, 2, 2, W], f16)     # hidden lhsT chunks
            for li in range(2):
                for k in range(2):
                    nc.sync.dma_start(out=wh_s[:, li, k, :], in_=wh_d[li, k])
            wout_s = const.tile([128, 2, D], f16)      # W_out.T chunks
            for k in range(2):
                nc.sync.dma_start(out=wout_s[:, k, :], in_=wout_d[k])
            if not zero_bias:
                bias_s = const.tile([128, 3, 2], f32)
                for j in range(3):
                    nc.sync.dma_start(out=bias_s[:, j, :], in_=bias_d[j])

            # ---- initial state: y (fp32) + fp16 shadow for matmuls ----
            ycur, yhcur = [], []
            for c in chunks:
                y_t = st.tile([D, csz], f32, tag=f"y{c}")
                nc.sync.dma_start(out=y_t[:], in_=y0_d[:, c * csz:(c + 1) * csz])
                yh_t = st.tile([D, csz], f16, tag=f"yh{c}")
                nc.sync.dma_start(out=yh_t[:], in_=y0h_d[:, c * csz:(c + 1) * csz])
                ycur.append(y_t)
                yhcur.append(yh_t)

            def tanh_layer(h_sb, h_ps, li):
                if zero_bias:
                    nc.scalar.activation(out=h_sb[:, :, :], in_=h_ps[:, :, :],
                                         func=Tanh)
                else:
                    for m in range(2):
                        nc.scalar.activation(
                            out=h_sb[:, m, :], in_=h_ps[:, m, :], func=Tanh,
                            bias=bias_s[:, li, m:m + 1], scale=1.0)

            # ---- time stepping (chunks interleaved per stage so each
            #      engine's in-order stream never stalls on the other's) ----
            for t in range(steps):
                dt = float(dts[t])
                nz_t, vt, h1p, h1, h2p, h2, h3p, h3, yp = ({} for _ in range(9))
                for c in chunks:
                    lo = c * csz
                    nz_t[c] = nzp.tile([D, csz], f32, tag=f"nz{c}", name=f"nz_{c}")
                    nc.sync.dma_start(out=nz_t[c][:], in_=nz_d[t, :, lo:lo + csz])

                for c in chunks:   # layer 1: [64 -> 256]
                    h1p[c] = ps.tile([128, 2, csz], f32, tag=f"h1p{c}", name=f"h1p_{c}")
                    mm(h1p[c][:, 0, :], win_s[:, 0:128], yhcur[c][:],
                       start=True, stop=True)
                    mm(h1p[c][:, 1, :], win_s[:, 128:256], yhcur[c][:],
                       start=True, stop=True)
                for c in chunks:
                    # v = (1-dt)*y + noise, early: off the L4 -> L1' chain
                    vt[c] = st.tile([D, csz], f32, tag=f"v{c}", name=f"v_{c}")
                    nc.vector.scalar_tensor_tensor(
                        out=vt[c][:], in0=ycur[c][:], scalar=1.0 - dt,
                        in1=nz_t[c][:], op0=MULT, op1=ADD)
                for c in chunks:
                    h1[c] = hbuf.tile([128, 2, csz], f16, tag=f"h1{c}", name=f"h1_{c}")
                    tanh_layer(h1[c], h1p[c], 0)

                for li, (hp, hs, hprev) in enumerate(
                        ((h2p, h2, h1), (h3p, h3, h2))):
                    for c in chunks:   # hidden layer: [256 -> 256]
                        hp[c] = ps.tile([128, 2, csz], f32, tag=f"h{li + 2}p{c}", name=f"h{li + 2}p_{c}")
                        for m in range(2):
                            for k in range(2):
                                mm(hp[c][:, m, :],
                                   wh_s[:, li, k, m * 128:(m + 1) * 128],
                                   hprev[c][:, k, :], start=(k == 0),
                                   stop=(k == 1))
                    for c in chunks:
                        hs[c] = hbuf.tile([128, 2, csz], f16, tag=f"h{li + 2}{c}", name=f"h{li + 2}_{c}")
                        tanh_layer(hs[c], hp[c], li + 1)

                for c in chunks:   # output layer: vf_psum = Wout @ H3
                    yp[c] = ps.tile([D, csz], f32, tag=f"yp{c}", name=f"yp_{c}")
                    mm(yp[c][:], wout_s[:, 0, :], h3[c][:, 0, :],
                       start=True, stop=False)
                    mm(yp[c][:], wout_s[:, 1, :], h3[c][:, 1, :],
                       start=False, stop=True)

                for c in chunks:
                    # y' = dt * vf_psum + v, produced twice: fp16 shadow first
                    # (feeds next step's matmuls ASAP), fp32 state second.
                    yh_nx = st.tile([D, csz], f16, tag=f"yh{c}", name=f"yh_{c}")
                    nc.vector.scalar_tensor_tensor(
                        out=yh_nx[:], in0=yp[c][:], scalar=dt, in1=vt[c][:],
                        op0=MULT, op1=ADD)
                    yhcur[c] = yh_nx
                for c in chunks:
                    lo = c * csz
                    y_nx = st.tile([D, csz], f32, tag=f"y{c}", name=f"y_{c}")
                    nc.vector.scalar_tensor_tensor(
                        out=y_nx[:], in0=yp[c][:], scalar=dt, in1=vt[c][:],
                        op0=MULT, op1=ADD)
                    nc.sync.dma_start(out=out_d[t, :, lo:lo + csz], in_=y_nx[:])
                    ycur[c] = y_nx
    nc.compile()
    return nc


def _host_prep(ts, y0, dW, w_in, b_in, w_h, b_h, w_out, b_out):
    """Compute per-core input maps + build params. Returns (in_maps, dts, zero_bias)."""
    f = np.float32
    ts = np.asarray(ts, f)
    dts = (ts[1:] - ts[:-1]).astype(f)
    assert dts.shape[0] == T

    zero_bias = (not np.any(b_in)) and (not np.any(b_h))

    # noise folded with its scale (and b_out drift term): sigma*sqrt(dt)*dW + dt*b_out
    scale = (SIGMA * np.sqrt(dts)).astype(f)  # [T]
    drift = (dts[:, None] * np.asarray(b_out, f)[None, :]).astype(f)  # [T, D]

    h = np.float16
    w_inT = np.ascontiguousarray(np.asarray(w_in, f).T.astype(h))    # [64, 256]
    whT = np.ascontiguousarray(
        np.stack([np.asarray(w_h[i], f).T.reshape(2, 128, W) for i in range(2)])
    ).astype(h)                                                      # [2, 2, 128, 256]
    w_outT = np.ascontiguousarray(np.asarray(w_out, f).T.reshape(2, 128, D)).astype(h)
    biases = np.zeros((3, 128, 2), f)
    biases[0] = np.asarray(b_in, f).reshape(2, 128).T
    biases[1] = np.asarray(b_h[0], f).reshape(2, 128).T
    biases[2] = np.asarray(b_h[1], f).reshape(2, 128).T

    y0 = np.asarray(y0, f)
    dW = np.asarray(dW, f)

    in_maps = []
    for c in range(NCORES):
        lo = c * BL
        nzc = dW[:, lo:lo + BL, :] * scale[:, None, None] + drift[:, None, :]
        nzc = np.ascontiguousarray(nzc.transpose(0, 2, 1)).astype(f)  # [T, 64, BL]
        y0tc = np.ascontiguousarray(y0[lo:lo + BL].T)
        m = {
            "y0t": y0tc,
            "y0th": y0tc.astype(np.float16),
            "nz": nzc,
            "wint": w_inT,
            "wht": whT,
            "woutt": w_outT,
        }
        if not zero_bias:
            m["biases"] = biases
        in_maps.append(m)
    return in_maps, dts, zero_bias


_NC_CACHE = {}

# test-harness hooks (kernel() ignores these unless set by test code)
TRACE = False
LAST_RESULT = None


def kernel(ts, y0, dW, w_in, b_in, w_h, b_h, w_out, b_out):
    global LAST_RESULT
    from concourse.bass_utils import run_bass_kernel_spmd

    in_maps, dts, zero_bias = _host_prep(
        ts, y0, dW, w_in, b_in, w_h, b_h, w_out, b_out)

    key = (zero_bias, np.asarray(dts).tobytes())
    nc = _NC_CACHE.get(key)
    if nc is None:
        nc = _build(dts, zero_bias)
        _NC_CACHE[key] = nc

    res = run_bass_kernel_spmd(nc, in_maps, core_ids=list(range(NCORES)),
                               trace=TRACE)
    LAST_RESULT = res

    out = np.empty((T + 1, B, D), np.float32)
    out[0] = np.asarray(y0, np.float32)
    for c in range(NCORES):
        lo = c * BL
        out[1:, lo:lo + BL, :] = res.results[c]["outt"].transpose(0, 2, 1)
    return out
